# revision 13
# baseline (speedup 1.0000x reference)
"""Trainium2 Bass kernel for the scatter_memory delta-rule module, v4.

Computation (per batch b, head h):
  Y = X @ [W_mk|W_mv|W_mb].T            (X = mem_tokens[b], [S, D])
  k_raw, new_mv, mb_raw = per-head 64-col slices of Y
  xx  = [relu(k), relu(-k)]             ([S, 128])
  mk_j = xx * roll_j(xx), j=1..3        (mk = [S, 384], all >= 0)
  ss  = ||mk||^2, r = sqrt(ss), alpha = 1/r
  num = mk @ W_mem, zmk = mk @ z        (retrieval)
  prev = num / (zmk + 1e-5*r)
  mvg = (new_mv - prev) * sigmoid(mb_raw) * alpha
  dW  = mk.T @ mvg ;  out = W_mem + dW

Key structure (vs the v1 baseline that DMA-transposed mk 24x/tile):
- mkT (k-major, for the retrieval stationary operand) is built from
  8 PE transposes of xx per tile plus 3 partition-shifted SBUF->SBUF
  DMA copies of xxT; mkT_j = xxT * shift_j(xxT) on the vector engine.
- The cyclic wrap rows (t < j) of the shifts would need 1-3-partition
  DMAs (descriptor-sprayed, ~4us each), so instead those rows stay 0
  (one-time memset of two persistent ping-pong shift tiles) and the
  missing retrieval contribution is added by a per-head correction
  matmul whose 12-row stationary comes from one PE transpose of the
  s-major mk wrap columns (host packs the matching wrap weights).
- The rsqrt/beta scalar stage is batched over 4 tiles (small-op
  overhead dominates at FD=8).
- PSUM: 4 persistent dW banks + one 4-slot rotating pool
  (psK/psV/psB/xxT/mkw/psR halves); psV and psR are evacuated to
  SBUF early by the scalar engine.

Sharding: 8 cores = (4 batches) x (2 half-head groups of 8 heads).
Device returns dW.T [H, 64, 384] fp32; host transposes and adds W_mem.
"""

import numpy as np
import ml_dtypes
from contextlib import ExitStack


def _split_excess_waits(nc, max_waits=1, drain_waits=1):
    """The walrus build here encodes only ONE sync wait per instruction
    (updates are separate). Move excess waits onto prepended same-engine
    drains, one wait each."""
    from concourse import mybir

    ctr = [0]
    for f in nc.m.functions:
        for bb in f.blocks:
            il = list(bb.instructions)
            out = []
            changed = False
            for inst in il:
                si = getattr(inst, "sync_info", None)
                waits = list(si.on_wait) if si and si.on_wait else []
                ups = list(si.on_update) if si and si.on_update else []
                if len(waits) > max_waits:
                    keep = waits[:max_waits]
                    rest = waits[max_waits:]
                    for i in range(0, len(rest), drain_waits):
                        chunk = rest[i:i + drain_waits]
                        ctr[0] += 1
                        d = mybir.InstDrain(
                            name=f"waitsplit{ctr[0]}",
                            ins=[],
                            outs=[],
                            bass_is_fusable=False,
                        )
                        d.engine = inst.engine
                        d.sync_info = mybir.SyncInfo(on_wait=chunk, on_update=[])
                        out.append(d)
                    inst.sync_info = mybir.SyncInfo(on_wait=keep, on_update=ups)
                    changed = True
                out.append(inst)
            if changed:
                bb.instructions = out
    return ctr[0]


B, S, D = 4, 4096, 1024
HPC = 8            # heads per core
NCORES = 8
DK = 64            # dk per head
DKEY = 384         # 2*nu*dk
DV = 64
ST = 128           # tokens per tile
NST = S // ST      # 32
NJ = 3
GRP = 4            # tiles per scalar-stage batch


def _body(ctx, tc, out_dwt, xt, wt, rhs, perm, ident, nst):
    import concourse.bass as bass
    from concourse import mybir

    nc = tc.nc
    bf16 = mybir.dt.bfloat16
    f32 = mybir.dt.float32
    i32 = mybir.dt.int32
    Alu = mybir.AluOpType
    Act = mybir.ActivationFunctionType

    singles = ctx.enter_context(tc.tile_pool(name="singles", bufs=1))
    xpool = ctx.enter_context(tc.tile_pool(name="xpool", bufs=4))
    work = ctx.enter_context(tc.tile_pool(name="work", bufs=3))
    hold = ctx.enter_context(tc.tile_pool(name="hold", bufs=2 * GRP + 1))
    tiny = ctx.enter_context(tc.tile_pool(name="tiny", bufs=2))
    xx2pool = ctx.enter_context(tc.tile_pool(name="xx2pool", bufs=3))
    rot = ctx.enter_context(tc.tile_pool(name="rot", bufs=3, space="PSUM"))
    rpool = ctx.enter_context(tc.tile_pool(name="rpool", bufs=1, space="PSUM"))
    dpool = ctx.enter_context(tc.tile_pool(name="dpool", bufs=1, space="PSUM"))

    # ---- resident weights ----
    wt_sb = singles.tile([128, 8, 3 * HPC * DK], bf16)   # [p, dchunk, 1536]
    wt_r = wt.rearrange("(c p) f -> p c f", p=128)
    for d in range(8):
        for wv in range(3):
            nc.sync.dma_start(
                out=wt_sb[:, d, wv * 512:(wv + 1) * 512],
                in_=wt_r[:, d, wv * 512:(wv + 1) * 512],
            )
    rhs_sb = singles.tile([128, HPC, NJ, 65], bf16)      # [klow, h, j, 65]
    rhs_r = rhs.rearrange("h j p c -> p h j c")
    for h in range(HPC):
        for j in range(NJ):
            nc.sync.dma_start(out=rhs_sb[:, h, j, :], in_=rhs_r[:, h, j, :])
    ident_sb = singles.tile([128, 128], bf16)
    nc.sync.dma_start(out=ident_sb, in_=ident)
    perm_sb = singles.tile([128, NJ, 128], bf16)         # cyclic-shift matrices
    nc.sync.dma_start(out=perm_sb, in_=perm.rearrange("j p c -> p j c"))

    # persistent dW.T accumulators: 4 psum tiles, 2 heads each ([0:64],[64:128]).
    # Zeroed once; all outer MMs run with start=False so per-element
    # has_written bits give accumulate semantics without bank-level groups.
    dw_ps = [
        dpool.tile([128, DKEY], f32, tag=f"dw{i}", name=f"dw{i}") for i in range(4)
    ]
    for i in range(4):
        nc.vector.memset(dw_ps[i], 0.0)

    assert nst % GRP == 0
    ngrp = nst // GRP

    def tiny_stage(ss4):
        """Batched rsqrt/beta inputs for one group; returns (r5, alpha_b)."""
        t0 = tiny.tile([128, GRP, HPC], f32, tag="t0")
        nc.vector.tensor_scalar(t0, ss4, 1e-20, None, op0=Alu.max)
        yv = tiny.tile([128, GRP, HPC], f32, tag="yv")
        sh = tiny.tile([128, GRP, HPC], f32, tag="sh")
        nc.vector.tensor_scalar(
            sh.bitcast(i32), t0.bitcast(i32), 1, None,
            op0=Alu.logical_shift_right,
        )
        nc.vector.tensor_scalar(
            yv.bitcast(i32), sh.bitcast(i32), -1, 0x5F3759DF,
            op0=Alu.mult, op1=Alu.add,
        )
        aa = tiny.tile([128, GRP, HPC], f32, tag="aa")
        bb = tiny.tile([128, GRP, HPC], f32, tag="bb")
        for _ in range(2):
            nc.vector.tensor_tensor(aa, yv, yv, op=Alu.mult)
            nc.vector.tensor_tensor(bb, aa, t0, op=Alu.mult)
            nc.vector.tensor_scalar(bb, bb, -0.5, 1.5, op0=Alu.mult, op1=Alu.add)
            nc.vector.tensor_tensor(yv, yv, bb, op=Alu.mult)
        t5 = tiny.tile([128, GRP, HPC], f32, tag="t5")
        nc.vector.tensor_scalar(t5, t0, 1e-5, None, op0=Alu.mult)
        r5 = tiny.tile([128, GRP, HPC], f32, tag="r5")
        nc.vector.tensor_tensor(r5, t5, yv, op=Alu.mult)
        alpha_b = tiny.tile([128, GRP, HPC], bf16, tag="alphab")
        nc.vector.tensor_copy(alpha_b, yv)
        return r5, alpha_b

    def epilogue(i, held, r5, alpha_b):
        """Per-tile beta, mvg, outer-product accumulation."""
        num_sb, mv_sb, g_sb, mk = held
        d0 = tiny.tile([128, HPC], f32, tag="d0")
        nc.vector.tensor_tensor(
            d0, num_sb[:, :, 64], r5[:, i, :], op=Alu.add
        )
        beta = tiny.tile([128, HPC], f32, tag="beta")
        nc.vector.reciprocal(beta, d0)
        p1 = work.tile([128, HPC, DK], bf16, tag="p1")
        nc.vector.tensor_tensor(
            p1, num_sb[:, :, 0:64], beta.broadcast_to([128, HPC, DK]),
            op=Alu.mult,
        )
        m1 = work.tile([128, HPC, DK], bf16, tag="m1")
        nc.vector.tensor_tensor(m1, mv_sb, p1, op=Alu.subtract)
        gg = work.tile([128, HPC, DK], bf16, tag="gg")
        nc.vector.tensor_tensor(
            gg, g_sb, alpha_b[:, i, :].broadcast_to([128, HPC, DK]),
            op=Alu.mult,
        )
        mvg = work.tile([128, HPC, DK], bf16, tag="mvg")
        nc.vector.tensor_tensor(mvg, m1, gg, op=Alu.mult)
        for h in range(HPC):
            nc.tensor.matmul(
                dw_ps[h // 2][64 * (h % 2):64 * (h % 2) + 64, :],
                mvg[:, h, :],
                mk[:, h, :, :],
                start=False,
                stop=False,
                skip_group_check=True,
                tile_position=(0, 64 * (h % 2)),
            )

    def a2_stage(xx2, held):
        """Transposes, shifted copies, mkT products, retrieval for one tile."""
        xxT = work.tile([128, HPC, 128], bf16, tag="xxT")   # [t, h, s]
        xxT_ps = rot.tile([128, HPC, 128], bf16, tag="rot")
        for h in range(HPC):
            nc.tensor.transpose(xxT_ps[:, h, :], xx2[:, h, 0:128], ident_sb)
        nc.scalar.activation(xxT, xxT_ps, Act.Copy)
        xxT_f = xxT.rearrange("p h s -> p (h s)")
        mkT = work.tile([128, NJ, HPC, 128], bf16, tag="mkT")
        for j in range(NJ):
            for half in range(2):
                ps_sh = rot.tile([128, 4, 128], f32, tag="rot")
                nc.tensor.matmul(
                    ps_sh.rearrange("p h s -> p (h s)"),
                    perm_sb[:, j, :],
                    xxT_f[:, half * 512:half * 512 + 512],
                    start=True, stop=True,
                )
                nc.vector.tensor_tensor(
                    mkT[:, j, half * 4:half * 4 + 4, :],
                    xxT[:, half * 4:half * 4 + 4, :],
                    ps_sh, op=Alu.mult,
                )
        num_sb = hold.tile([128, HPC, 65], bf16, tag="num")
        for half in range(2):
            psR = rpool.tile([128, 4, 65], f32, tag="psr")
            for hh in range(4):
                h = half * 4 + hh
                for j in range(NJ):
                    nc.tensor.matmul(
                        psR[:, hh, 0:65],
                        mkT[:, j, h, :],
                        rhs_sb[:, h, j, :],
                        start=(j == 0),
                        stop=(j == NJ - 1),
                    )
            nc.vector.tensor_copy(
                num_sb[:, half * 4:half * 4 + 4, :], psR
            )
        held[0] = num_sb

    prev_held, prev_ss4 = None, None
    pend_a2 = None
    for grp in range(ngrp):
        held_t = []
        ss4 = tiny.tile([128, GRP, HPC], f32, tag="ss4")
        pr5 = palpha = None
        if prev_held is not None:
            pr5, palpha = tiny_stage(prev_ss4)
        for i in range(GRP):
            if prev_held is not None:
                epilogue(i, prev_held[i], pr5, palpha)
            st = grp * GRP + i
            s0 = st * ST
            # ---- load X.T tile (one DMA, 3D AP) ----
            x_sb = xpool.tile([128, 8, ST], bf16)
            xt_r = xt[:, s0:s0 + ST].rearrange("(c p) s -> p c s", p=128)
            nc.sync.dma_start(out=x_sb, in_=xt_r)

            # ---- projections: three waves of 8 accumulating matmuls ----
            psK = rot.tile([128, 512], f32, tag="rot", name=f"psK{st}")
            for d in range(8):
                nc.tensor.matmul(
                    psK, x_sb[:, d, :], wt_sb[:, d, 0:512],
                    start=(d == 0), stop=(d == 7),
                )

            # ---- relus -> xx2 (duplicated [xx | xx]) ----
            xx2 = xx2pool.tile([128, HPC, 256], bf16, tag="xx2")
            kin = psK.rearrange("p (h f) -> p h f", h=HPC)
            for neg, off in ((False, 0), (True, 64)):
                dst = bass.AP(
                    tensor=xx2.tensor,
                    offset=xx2.offset + off,
                    ap=[xx2.ap[0], [256, HPC], [128, 2], [1, 64]],
                )
                src = bass.AP(
                    tensor=kin.tensor,
                    offset=kin.offset,
                    ap=[kin.ap[0], [64, HPC], [0, 2], [1, 64]],
                )
                if neg:
                    nc.scalar.activation(dst, src, Act.Relu, scale=-1.0)
                else:
                    nc.scalar.activation(dst, src, Act.Relu)

            psV = rot.tile([128, 512], f32, tag="rot", name=f"psV{st}")
            for d in range(8):
                nc.tensor.matmul(
                    psV, x_sb[:, d, :], wt_sb[:, d, 512:1024],
                    start=(d == 0), stop=(d == 7),
                )
            psB = rot.tile([128, 512], f32, tag="rot", name=f"psB{st}")
            for d in range(8):
                nc.tensor.matmul(
                    psB, x_sb[:, d, :], wt_sb[:, d, 1024:1536],
                    start=(d == 0), stop=(d == 7),
                )

            # ---- evacuate psV / sigmoid psB early (frees rot slots) ----
            mv_sb = hold.tile([128, HPC, DK], bf16, tag="mv")
            nc.scalar.activation(
                mv_sb, psV.rearrange("p (h f) -> p h f", h=HPC), Act.Copy
            )
            g_sb = hold.tile([128, HPC, DK], bf16, tag="g")
            nc.scalar.activation(
                g_sb, psB.rearrange("p (h f) -> p h f", h=HPC), Act.Sigmoid
            )

            # ---- phi products s-major: mk_j[t] = xx[t]*xx[t-j] ----
            mk = hold.tile([128, HPC, NJ, 128], bf16, tag="mk")
            xx_c = xx2[:, :, 128:256]
            nc.vector.tensor_tensor(
                mk[:, :, 1, :], xx_c, xx2[:, :, 126:254], op=Alu.mult
            )  # j=2
            nc.vector.tensor_tensor(
                mk[:, :, 0, :], xx_c, xx2[:, :, 127:255], op=Alu.mult
            )  # j=1
            nc.vector.tensor_tensor(
                mk[:, :, 2, :], xx_c, xx2[:, :, 125:253], op=Alu.mult
            )  # j=3


            # ---- ss = sum(mk^2) via u*v window trick ----
            u2 = work.tile([128, HPC, 131], bf16)
            nc.scalar.activation(u2[:, :, 3:131], xx_c, Act.Square)
            nc.scalar.activation(u2[:, :, 0:3], xx2[:, :, 253:256], Act.Square)
            v_sb = work.tile([128, HPC, 128], bf16)
            nc.gpsimd.tensor_tensor(
                v_sb, u2[:, :, 2:130], u2[:, :, 1:129], op=Alu.add
            )
            nc.gpsimd.tensor_tensor(v_sb, v_sb, u2[:, :, 0:128], op=Alu.add)
            w_sb = work.tile([128, HPC, 128], bf16)
            nc.gpsimd.tensor_tensor(w_sb, u2[:, :, 3:131], v_sb, op=Alu.mult)
            nc.vector.tensor_reduce(
                ss4[:, i, :], w_sb, axis=mybir.AxisListType.X, op=Alu.add
            )

            held = [None, mv_sb, g_sb, mk]
            held_t.append(held)
            if pend_a2 is not None:
                a2_stage(*pend_a2)
            pend_a2 = (xx2, held)

        prev_held, prev_ss4 = held_t, ss4

    # drain the pending a2 and the last group's epilogue
    a2_stage(*pend_a2)
    pr5, palpha = tiny_stage(prev_ss4)
    for i in range(GRP):
        epilogue(i, prev_held[i], pr5, palpha)

    # ---- write out dW.T (PSUM -> SBUF -> DRAM) ----
    for i in range(4):
        dwsb = work.tile([128, DKEY], f32, tag="dwsb", name=f"dwsb{i}")
        nc.vector.tensor_copy(dwsb, dw_ps[i])
        nc.sync.dma_start(
            out=out_dwt[2 * i:2 * i + 2].rearrange("h v k -> (h v) k"),
            in_=dwsb,
        )


def _build(nst=NST, split_waits=True):
    import concourse.bass as bass
    import concourse.tile as tile
    from concourse import mybir

    nc = bass.Bass(trn_type="TRN2", num_devices=NCORES)
    xt = nc.dram_tensor("xt", (D, S), mybir.dt.bfloat16, kind="ExternalInput").ap()
    wt = nc.dram_tensor(
        "wt", (D, 3 * HPC * DK), mybir.dt.bfloat16, kind="ExternalInput"
    ).ap()
    rhs = nc.dram_tensor(
        "rhs", (HPC, NJ, 128, 65), mybir.dt.bfloat16, kind="ExternalInput"
    ).ap()
    perm = nc.dram_tensor(
        "perm", (NJ, 128, 128), mybir.dt.bfloat16, kind="ExternalInput"
    ).ap()
    ident = nc.dram_tensor(
        "ident", (128, 128), mybir.dt.bfloat16, kind="ExternalInput"
    ).ap()
    out = nc.dram_tensor(
        "dwt", (HPC, DV, DKEY), mybir.dt.float32, kind="ExternalOutput"
    ).ap()
    with tile.TileContext(nc) as tc:
        with ExitStack() as ctx:
            _body(ctx, tc, out, xt, wt, rhs, perm, ident, nst)
    if split_waits:
        n = _split_excess_waits(nc)
        print(f"[kernel] split {n} excess-wait chunks onto drains")
    return nc


_CACHE = {}


def _prep_core_inputs(mem_tokens, W_mk, W_mv, W_mb, W_mem, z):
    """Build the 8 per-core input maps (host-side shard + layout prep)."""
    bf = ml_dtypes.bfloat16
    eye = np.eye(128, dtype=np.float32).astype(bf)
    PERM = np.zeros((NJ, 128, 128), dtype=np.float32)
    for j in range(NJ):
        for t in range(128):
            PERM[j, t, (t + j + 1) % 128] = 1.0
    PERM = PERM.astype(bf)
    in_maps = []
    for c in range(NCORES):
        b = c // 2
        h0 = (c % 2) * HPC
        xt = np.ascontiguousarray(mem_tokens[b].T).astype(bf)        # [D, S]
        ws = []
        for W in (W_mk, W_mv, W_mb):
            ws.append(W[h0 * DK:(h0 + HPC) * DK, :])                 # [512, D]
        wt = np.ascontiguousarray(np.concatenate(ws, axis=0).T).astype(bf)
        rhs = np.zeros((HPC, NJ, 128, 65), dtype=np.float32)
        wm = W_mem[b, h0:h0 + HPC]                                   # [8, 384, 64]
        zz = z[b, h0:h0 + HPC]                                       # [8, 384]
        for j in range(NJ):
            rhs[:, j, :, 0:64] = wm[:, j * 128:(j + 1) * 128, :]
            rhs[:, j, :, 64] = zz[:, j * 128:(j + 1) * 128]
        in_maps.append(
            {"xt": xt, "wt": wt, "rhs": rhs.astype(bf),
             "perm": PERM, "ident": eye}
        )
    return in_maps


def kernel(mem_tokens, W_mk, W_mv, W_mb, W_mem, z, _want_profile=False):
    from concourse.bass_utils import run_bass_kernel_spmd

    if "nc" not in _CACHE:
        _CACHE["nc"] = _build()
    nc = _CACHE["nc"]
    in_maps = _prep_core_inputs(mem_tokens, W_mk, W_mv, W_mb, W_mem, z)
    res = run_bass_kernel_spmd(
        nc, in_maps, core_ids=list(range(NCORES)), trace=_want_profile
    )
    out = np.empty((B, 16, DKEY, DV), dtype=np.float32)
    for c in range(NCORES):
        b = c // 2
        h0 = (c % 2) * HPC
        dwt = np.asarray(res.results[c]["dwt"]).reshape(HPC, DV, DKEY)
        out[b, h0:h0 + HPC] = np.transpose(dwt, (0, 2, 1))
    out += W_mem.astype(np.float32)
    if _want_profile:
        return out, res
    return out
